# revision 9
# baseline (speedup 1.0000x reference)
"""Multi-head attention (B=2, T=4096, H=8, D=64, non-causal full softmax)
for 8 Trainium2 NeuronCores.

Sharding: 16 (batch, head-pair) units -> core c handles batch c//4 and the
head pair (2*(c%4), 2*(c%4)+1).

Per-core pipeline (engine assignments chosen so the two PSUM-capable lane
engines, ACT and DVE, spend nearly all their time on the softmax exp):
  1. x chunks DMA in (fp32); GPSIMD converts fp32->f16 (SBUF->SBUF, the
     only PSUM-free engine); HWDGE DMA-transpose (XBAR) produces
     xT [128, 4, T] f16 with zero PE/lane cost.
  2. PE projects qT/kT [128, T] f16 (two heads stacked on partitions;
     f16 keeps the score error out of the softmax tail) and
     v -> va [128, 16, 2, 160] fp8 (per jpair/chunk-parity/head, padded to
     ktile stride 80|160 for the DoubleRow %16 rule, with a ones column at
     64|144 so the O matmul also emits softmax denominators).  DVE drains
     PSUM->SBUF (q/k add their per-partition biases via tensor_scalar).
  3. Attention: S^T tiles [128, 1024] per (chunk-pair, head) via f16
     matmuls (contraction d=64, tile_position=(64h, 0)).  exp is an
     AFFINE: the e4m3 BIT PATTERN of exp(s/8 - C) is
     round(s/(8 ln2)*8 + B) saturated to uint8 - computed by ACT
     (activation Copy w/ scale+bias) or DVE (tensor_scalar) directly from
     PSUM, split ~55/45.  O''^T accumulates PSUM-resident across all
     groups via fp8 DoubleRow matmuls (ktiles = chunk pair).
  4. Epilogue: ACT copies ps_o [65, 1024] -> f16, DMA to DRAM.  The host
     transposes, divides by the denominator row, and adds bv (bv and the
     1/denominator normalization commute with the head-dim transpose).
Softmax shift C cancels row-wise (uniform across all tiles); Schraudolph
centering also cancels.  exp(s/8 - C) stays within e4m3 range for the
observed score range s/8 in [-6.94, 8.01] with margin to 9.0.
"""

import numpy as np

T = 4096
DM = 512
N_CORES = 8
NI = T // 512    # 8 i-tiles
NJP = T // 256   # 16 chunk pairs
LN2 = float(np.log(2.0))

# exp affine: f16 bitpattern of exp(0.125*s - C) = round(A16*s + B16) (int16)
C_SHIFT = 2.8
KAPPA = -0.0596
A16 = 0.125 * 1024.0 / LN2
B16 = 15360.0 + 1024.0 * (-C_SHIFT / LN2 + KAPPA)

CONFIG = dict(
    exp_frac_act=0.55,   # fraction of exp tiles on the scalar (ACT) engine
)

_CACHE = {}


def _split_multi_waits(nc, limit=1):
    """The walrus build in this container encodes at most `limit` sem-waits
    per instruction (any engine).  Move extra waits onto same-engine NoOps
    inserted immediately before the instruction (semantically identical:
    the engine stream executes them in order)."""
    import bass_rust
    import concourse.mybir as mybir

    uid = 0
    for f in nc.m.functions:
        for bb in f.blocks:
            insts = bb.instructions
            new_list = []
            for inst in insts:
                si = inst.sync_info
                if si is not None and len(si.on_wait) > limit:
                    waits = list(si.on_wait)
                    ups = list(si.on_update)
                    for w in waits[:-limit]:
                        uid += 1
                        n = mybir.InstNoOp(name=f"wsplit-{uid}", engine=inst.engine)
                        n.sync_info = bass_rust.SyncInfo(on_wait=[w], on_update=[])
                        new_list.append(n)
                    inst.sync_info = bass_rust.SyncInfo(
                        on_wait=waits[-limit:], on_update=ups
                    )
                new_list.append(inst)
            insts[:] = new_list
    return nc


def build_program(exp_frac_act=0.55):
    import concourse.bass as bass
    import concourse.mybir as mybir
    from concourse.tile import TileContext, ScopedClock
    from contextlib import ExitStack

    class _LeanTailTC(TileContext):
        """Skip the exit barriers + semaphore clears (~10us EVSEM storm):
        the runtime re-zeroes semaphores per execution (verified by
        repeat-run correctness), so the final drain + its waits suffice."""
        def _drain_and_barrier(self, tick_clock, wait_clock):
            drain_inst = self.nc.sync.drain()
            wait_clock.add_sem_waits(
                drain_inst.ins, ScopedClock({None: tick_clock.global_clock}))
            popped = self.nc._tile_sem_poison_stack.pop()
            assert popped is self._sem_poison

    fp32 = mybir.dt.float32
    f16 = mybir.dt.float16
    fp8 = mybir.dt.float8e4
    u8 = mybir.dt.uint8
    i16 = mybir.dt.int16
    AF = mybir.ActivationFunctionType
    DR = mybir.MatmulPerfMode.DoubleRow

    nc = bass.Bass("TRN2", num_devices=N_CORES)
    x_d = nc.declare_dram_parameter("x", [T, DM], f16, isOutput=False)
    wq_d = nc.declare_dram_parameter("wq", [DM, 128], f16, isOutput=False)
    wk_d = nc.declare_dram_parameter("wk", [DM, 128], f16, isOutput=False)
    wv_d = nc.declare_dram_parameter("wv", [DM, 128], f16, isOutput=False)
    bq_d = nc.declare_dram_parameter("bq", [1, 128], fp32, isOutput=False)
    bk_d = nc.declare_dram_parameter("bk", [1, 128], fp32, isOutput=False)
    out_d = nc.declare_dram_parameter("out", [NI * 65, 1024], f16, isOutput=True)

    # exp tile engine assignment (Bresenham on exp_frac_act)
    n_tiles = NI * NJP * 2
    exp_engine = []
    accum = 0.0
    for _ in range(n_tiles):
        accum += exp_frac_act
        if accum >= 1.0:
            accum -= 1.0
            exp_engine.append("A")
        else:
            exp_engine.append("D")

    with _LeanTailTC(nc) as tc, ExitStack() as ctx:
        const = ctx.enter_context(tc.tile_pool(name="const", bufs=1))
        big = ctx.enter_context(tc.tile_pool(name="big", bufs=1))
        exp_pool = ctx.enter_context(tc.tile_pool(name="exp", bufs=8))
        accp = ctx.enter_context(tc.tile_pool(name="accp", bufs=3))
        ps_s_pool = ctx.enter_context(tc.tile_pool(name="ps_s", bufs=3, space="PSUM"))
        ps_o_pool = ctx.enter_context(tc.tile_pool(name="ps_o", bufs=1, space="PSUM"))

        # --- warmup: PE clock (HAM) + first DMAs in flight ---------------
        warm = const.tile([128, 1], fp32)
        nc.gpsimd.memset(warm, 0.0)
        warm2 = const.tile([128, 1], fp32)
        nc.gpsimd.memset(warm2, 0.0)
        nc.scalar.activation(out=warm, in_=warm2, func=AF.Copy)
        ps_w = ps_s_pool.tile([1, 1], fp32, tag="s", name="ps_w")
        for _ in range(18):
            nc.tensor.matmul(ps_w, lhsT=warm2, rhs=warm2, start=True, stop=True)

        # --- params ------------------------------------------------------
        wq_sb = const.tile([128, 4, 128], f16, name="wq_sb")
        nc.sync.dma_start(out=wq_sb, in_=wq_d.ap().rearrange(
            "(c p) m -> p c m", p=128))
        wk_sb = const.tile([128, 4, 128], f16, name="wk_sb")
        nc.sync.dma_start(out=wk_sb, in_=wk_d.ap().rearrange(
            "(c p) m -> p c m", p=128))
        wv_sb = const.tile([128, 4, 128], f16, name="wv_sb")
        nc.sync.dma_start(out=wv_sb, in_=wv_d.ap().rearrange(
            "(c p) m -> p c m", p=128))
        # q/k biases are per-partition in the qT/kT layout
        bq_p = const.tile([128, 1], fp32, name="bq_p")
        nc.sync.dma_start(out=bq_p, in_=bq_d.ap().rearrange("o c -> c o"))
        bk_p = const.tile([128, 1], fp32, name="bk_p")
        nc.sync.dma_start(out=bk_p, in_=bk_d.ap().rearrange("o c -> c o"))

        # --- big persistent tiles ---------------------------------------
        xT = big.tile([128, 4, T], f16)       # xT[p, kc, t] = x[t, kc*128+p]
        qT = big.tile([128, T], f16)          # qT[c, t], c = 2 heads x 64 dims
        kT = big.tile([128, T], f16)
        va = big.tile([128, NJP, 2, 160], f16)  # [p, jp, par, 80h+ (64 v | 1 | pad)]
        nc.gpsimd.memset(va, 0.0)
        va_ones = bass.AP(tensor=va.tensor, offset=va.offset + 64,
                          ap=[va.ap[0], [320, NJP], [160, 2], [80, 2]])
        nc.gpsimd.memset(va_ones, 1.0)

        # --- prologue ----------------------------------------------------
        def produce(m):
            i0m = m * 512
            nc.sync.dma_start_transpose(
                out=xT[:, :, i0m:i0m + 512], in_=x_d.ap()[i0m:i0m + 512, :])

            i0 = m * 512
            for (w_sb, b_p, dstT) in ((wk_sb, bk_p, kT), (wq_sb, bq_p, qT)):
                ps_qk = ps_s_pool.tile([128, 512], fp32, tag="s", name="ps_qk")
                for kc in range(4):
                    nc.tensor.matmul(
                        ps_qk,
                        lhsT=w_sb[:, kc, :],
                        rhs=xT[:, kc, i0:i0 + 512],
                        start=(kc == 0), stop=(kc == 3),
                    )
                nc.vector.tensor_scalar_add(dstT[:, i0:i0 + 512], ps_qk, b_p)

            for jp in (2 * m, 2 * m + 1):
                ps_v2 = ps_s_pool.tile([128, 256], fp32, tag="s", name="ps_v2")
                for par in range(2):
                    t0 = jp * 256 + par * 128
                    for kc in range(4):
                        nc.tensor.matmul(
                            ps_v2[:, par * 128:(par + 1) * 128],
                            lhsT=xT[:, kc, t0:t0 + 128],
                            rhs=wv_sb[:, kc, :],
                            start=(kc == 0), stop=(kc == 3),
                        )
                va_dst = bass.AP(tensor=va.tensor, offset=va.offset + jp * 320,
                                 ap=[va.ap[0], [160, 2], [80, 2], [1, 64]])
                nc.vector.tensor_copy(
                    out=va_dst,
                    in_=ps_v2.rearrange("p (k h c) -> p k h c", k=2, h=2))

        # --- attention ---------------------------------------------------
        tile_idx = [0]

        def unit(it, g, ps_o):
            i0 = it * 512
            for jp in (2 * g, 2 * g + 1):
                # Emit the four S matmuls with alternating tile positions
                # (0,0)/(64,0): row-tiled pairs co-execute on the PE
                # (measured 3.2x vs same-position back-to-back).
                pss = [ps_s_pool.tile([128, 1024], fp32, tag="s", name="ps")
                       for _ in range(2)]
                for par in range(2):
                    j0 = (jp * 2 + par) * 128
                    for h in range(2):
                        nc.tensor.matmul(
                            pss[h][:, par * 512:(par + 1) * 512],
                            lhsT=kT[64 * h:64 * h + 64, j0:j0 + 128],
                            rhs=qT[64 * h:64 * h + 64, i0:i0 + 512],
                            start=True, stop=True,
                            tile_position=(64 * h, 0),
                        )
                for h in range(2):
                    es = exp_pool.tile([128, 1024], i16, tag="es", name="es")
                    if exp_engine[tile_idx[0]] == "A":
                        nc.scalar.activation(out=es, in_=pss[h], func=AF.Copy,
                                             scale=A16, bias=B16)
                    else:
                        nc.vector.tensor_scalar(es, pss[h], A16, B16,
                                                mybir.AluOpType.mult,
                                                mybir.AluOpType.add)
                    tile_idx[0] += 1
                    esh = es.bitcast(f16)
                    for par in range(2):
                        nc.tensor.matmul(
                            ps_o[:, 512 * h:512 * h + 512],
                            lhsT=va[:, jp, par, 80 * h:80 * h + 65],
                            rhs=esh[:, par * 512:(par + 1) * 512],
                            start=(g == 0 and jp == 0 and par == 0),
                            stop=(g == 7 and jp == 15 and par == 1),
                            skip_group_check=True,
                        )

        for m in range(NI):
            produce(m)
        for it in range(NI):
            ps_o = ps_o_pool.tile([65, 1024], fp32, tag="o", name="ps_o")
            for g in range(NI):
                unit(it, g, ps_o)
            acc_t = accp.tile([65, 1024], f16, tag="acc", name="acc_t")
            nc.scalar.activation(out=acc_t, in_=ps_o, func=AF.Copy)
            nc.sync.dma_start(out=out_d.ap()[it * 65:(it + 1) * 65, :], in_=acc_t)

    _split_multi_waits(nc)
    return nc


def _core_inputs(x, Wq, bq, Wk, bk, Wv, bv):
    ins = []
    for core in range(N_CORES):
        b, p = divmod(core, 4)
        c0 = 128 * p
        ins.append({
            "x": np.ascontiguousarray(x[b], dtype=np.float16),
            "wq": np.ascontiguousarray(Wq[:, c0:c0 + 128], dtype=np.float16),
            "wk": np.ascontiguousarray(Wk[:, c0:c0 + 128], dtype=np.float16),
            "wv": np.ascontiguousarray(Wv[:, c0:c0 + 128], dtype=np.float16),
            "bq": np.ascontiguousarray(bq[c0:c0 + 128].reshape(1, 128), dtype=np.float32),
            "bk": np.ascontiguousarray(bk[c0:c0 + 128].reshape(1, 128), dtype=np.float32),
        })
    return ins


def kernel(x, Wq, bq, Wk, bk, Wv, bv):
    from concourse.bass_utils import run_bass_kernel_spmd

    if "nc" not in _CACHE:
        _CACHE["nc"] = build_program(**CONFIG)
    nc = _CACHE["nc"]

    x = np.asarray(x, dtype=np.float32)
    bv = np.asarray(bv, dtype=np.float32)
    ins = _core_inputs(x, np.asarray(Wq), np.asarray(bq), np.asarray(Wk),
                       np.asarray(bk), np.asarray(Wv), bv)
    res = run_bass_kernel_spmd(nc, ins, list(range(N_CORES)))
    B = x.shape[0]
    out = np.empty((B, T, DM), dtype=np.float32)
    for core in range(N_CORES):
        b, p = divmod(core, 4)
        c0 = 128 * p
        o = np.asarray(res.results[core]["out"]).astype(np.float32)
        o = o.reshape(NI, 65, 2, 512)            # [it, row, h, ti]
        vals = o[:, 0:64, :, :] / o[:, 64:65, :, :]
        block = vals.transpose(0, 3, 2, 1).reshape(T, 128) + bv[c0:c0 + 128]
        out[b, :, c0:c0 + 128] = block
    return out
